# revision 41
# baseline (speedup 1.0000x reference)
"""Trainium2 Bass kernel for ConvTranspose4d (T: 3-tap valid conv; D/H/W:
stride-2 k=3 p=1 transposed conv). Self-contained: hardcoded shapes.

x: [1, 8, 8, 24, 48, 48] f32, weight: [8, 8, 3, 3, 3, 3] f32
out: [1, 8, 6, 47, 95, 95] f32

Strategy (8 NeuronCores, data-parallel over D):
  - Core j computes output od = 6j..6j+5 (core 7 drops od 47); needs input
    slices id0..id0+3 (id0 = min(3j, 20)).
  - Temporal 3-tap conv and D-axis stride-2 transposed conv fold into the
    matmul stationary operand as a banded weight matrix:
      lhsT[K=128=(slot4, cin8, id4), M=(fbit2 x 48=(cout8*6+od))]
    where slot = kt + fbit holds temporal plane 2i+slot of frame-pair i.
  - All I/O is bf16 (host converts): one flat input DRAM tensor
    [128, bands(1152) | 3 x plane(2401)], loaded in range-split pieces
    strictly serialized on the sync HWDGE queue (concurrent queues contend
    per-engine at packet granularity).  Bands live in their own SBUF tile
    (lhsT+rhs from one tensor costs ~40ns/MM in port conflicts).
  - N=512 warm-up matmuls open the HAM clock gate (K=8/8) before the real
    stream, which then runs gap-free at ~206ns/MM (N=480, warm floor).
  - H/W parities are 4 output classes (ph, pw); each accumulates 1/2/2/4
    shifted-view taps in PSUM (kh = ph - 2*dh + 1).
  - Staging is PARITY-PLANAR: each class region is contiguous per
    partition (DVE for ph=0, ACT for ph=1; final pair alternates).  Output
    streams out DURING compute: pairs 0/1 drain per ph-half as soon as the
    half is staged, pair 2 per class with the last class pre-drained at
    chunk 2, so the post-stream tail is only ~170 KB + completion.
"""
import numpy as np

COMPUTE = "bfloat16"

TAPS = {
    (0, 0): [(0, 0)],
    (0, 1): [(0, 0), (0, 1)],
    (1, 0): [(0, 0), (1, 0)],
    (1, 1): [(0, 0), (0, 1), (1, 0), (1, 1)],
}
TAP_LIST = [(ph, pw, dh, dw) for (ph, pw), tl in TAPS.items() for (dh, dw) in tl]
CHUNK_START = [0, 10, 20, 30, 40]
CHUNK_N = [10, 10, 10, 10, 8]
PAIRS = [(0, 1), (2, 3), (4,)]
# class -> (region offset in stg, rows, cols)
REGION = {
    (0, 0): (0, 48, 48),
    (0, 1): (2304, 48, 47),
    (1, 0): (4560, 47, 48),
    (1, 1): (6816, 47, 47),
}

_CACHE = {}


def _bf16():
    import ml_dtypes
    return ml_dtypes.bfloat16


def _build_bands(W, j):
    """W: [cin8, cout8, kt3, kd3, kh3, kw3] -> [128, 9, 128] f32.
    K row = slot*32 + cin*4 + id (slot = kt + fbit);
    M col = fbit*48 + cout*6 + od (od 0..5; cols 96..127 zero)."""
    id0 = min(3 * j, 20)
    B = np.zeros((128, 9, 128), np.float32)
    ci = np.arange(8)
    co = np.arange(8)
    for t, (ph, pw, dh, dw) in enumerate(TAP_LIST):
        kh = ph - 2 * dh + 1
        kw = pw - 2 * dw + 1
        for fbit in range(2):
            for kt in range(3):
                slot = kt + fbit
                for idl in range(4):
                    for od in range(6):
                        od_g = 6 * j + od
                        if od_g > 46:
                            continue
                        kd = od_g - 2 * (id0 + idl) + 1
                        if not (0 <= kd <= 2):
                            continue
                        krow = slot * 32 + ci * 4 + idl
                        mcol = fbit * 48 + co * 6 + od
                        B[krow[:, None], t, mcol[None, :]] = W[:, :, kt, kd, kh, kw]
    return B


def _free_view(base, off, dims):
    """Hand-built AP: keep base's partition dim, replace free dims with
    [(step, count), ...] (element units) at extra offset `off`."""
    a = base.copy()
    v = a.ap
    part = v.to_list()[0]
    v.clear()
    v.append(part)
    for sc in dims:
        v.append(list(sc))
    a.ap = v
    a.offset = a.offset + off
    return a


def _build_program():
    import concourse.bacc as bacc
    import concourse.tile as tile
    from concourse import mybir

    f32 = mybir.dt.float32
    bf16 = mybir.dt.bfloat16

    nc = bacc.Bacc("TRN2", target_bir_lowering=False, debug=False)
    # flat per-partition layout [bands(1152) | p0(2401) | p1(2401) | p2(2401)]
    # -> one SBUF tile, big DMA descriptors, range-split loads so the data
    # matmul #0 needs (bands + plane-0 top rows) lands first.
    xs_ap = nc.dram_tensor("xs", [128, 8355], bf16, kind="ExternalInput").ap()
    # [pair, fbit, co*6+od, pos] -- matches stg partition order so each pair
    # drains as ONE big DMA (96 partitions x 18 KB).
    out_ap = nc.dram_tensor("out", [3, 2, 48, 9025], bf16, kind="ExternalOutput").ap()

    with tile.TileContext(nc, trace_sim=False) as tc:
        with (
            tc.tile_pool(name="bp", bufs=1) as bp,
            tc.tile_pool(name="sp", bufs=3) as sp,
            tc.tile_pool(name="ps", bufs=8, space="PSUM") as ps,
        ):
            # PE warm-up: N=512 dummy matmuls (~430ns each, ~100% PE-busy) so
            # the HAM clock-gate reaches K=8/8 before the real stream starts.
            # Small-N warmups measurably never warm it (too many dispatch
            # bubbles inside the 4096-cycle activity window).
            dz = bp.tile([128, 512], bf16)
            nc.gpsimd.memset(dz[:], 0.0)
            wps = ps.tile([128, 512], f32, name="warm", tag="ps")
            for _ in range(10):
                nc.tensor.matmul(wps[:, 0:512], dz[:, 0:128], dz[:],
                                 start=True, stop=True)

            # bands in their OWN SBUF tensor: lhsT and rhs from the same
            # tensor costs ~40ns/MM in SBUF port conflicts (measured).
            bt = bp.tile([128, 1152], bf16)
            xa = bp.tile([128, 7203], bf16)
            # Strictly serialize input on the sync HWDGE queue -- concurrent
            # queues contend per-engine at packet granularity (measured:
            # 301KB crawling at 50GB/s behind a parallel queue).  Order:
            # bands + plane-0 top rows (feeds MM0/chunks 0-1), rest of
            # plane 0, planes 1-2 (needed ~10us later).
            nc.sync.dma_start(out=bt[:], in_=xs_ap[:, 0:1152])
            nc.sync.dma_start(out=xa[:, 0:1225], in_=xs_ap[:, 1152:2377])
            nc.sync.dma_start(out=xa[:, 1225:2401], in_=xs_ap[:, 2377:3553])
            # 1-element WAW overlap with the previous piece: forces B to
            # start only after plane 0 has fully landed (otherwise their
            # descriptors interleave per-engine and plane 0 crawls).
            nc.sync.dma_start(out=xa[:, 2400:7203], in_=xs_ap[:, 3552:8355])

            for i in range(3):
                poff = i * 2401
                stg = sp.tile([128, 9025], bf16, name=f"stg{i}", tag="stg")
                corder = list(TAPS) if i < 2 else list(TAPS)[::-1]
                for (ph, pw) in corder:
                    taps = TAPS[(ph, pw)]
                    roff, _, nmw = REGION[(ph, pw)]
                    for c in range(5):
                        pt = ps.tile([128, 512], f32, name="ps", tag="ps")
                        mh0, nmh = CHUNK_START[c], CHUNK_N[c]
                        for ti, (dh, dw) in enumerate(taps):
                            t_idx = TAP_LIST.index((ph, pw, dh, dw))
                            lhsT = bt[:, t_idx * 128:(t_idx + 1) * 128]
                            rhs = _free_view(
                                xa[:], poff + (mh0 + dh) * 49 + dw,
                                [(49, nmh), (1, 48)])
                            nc.tensor.matmul(
                                pt[:, 0:nmh * 48], lhsT, rhs,
                                start=(ti == 0), stop=(ti == len(taps) - 1),
                            )
                        # contiguous copy PSUM -> class-planar staging; the
                        # final pair alternates DVE/ACT per chunk so copies
                        # chase the matmuls
                        nmh_c = nmh if c < 4 else CHUNK_N[4] - ph
                        src = _free_view(pt[0:96], 0, [(48, nmh_c), (1, nmw)])
                        doff = roff + CHUNK_START[c] * nmw
                        dst = _free_view(stg[0:96], doff,
                                         [(nmw, nmh_c), (1, nmw)])
                        use_dve = (c % 2 == 0) if i == 2 else (ph == 0)
                        if use_dve:
                            nc.vector.tensor_copy(dst, src)
                        else:
                            nc.scalar.copy(dst, src)
                        if i == 2 and (ph, pw) == (0, 0) and c == 2:
                            # pre-drain rows 0-29 of the very last class so
                            # the post-stream tail is only ~170 KB
                            nc.sync.dma_start(
                                out=out_ap[2, :, :, 0:1440],
                                in_=_free_view(stg[0:96], 0, [(1, 1440)]))
                    # Drain as soon as a region is staged, with the DMA
                    # issue placed before later copies in its engine's FIFO
                    # queue.  Pairs 0/1: per ph-half (the only two scalar
                    # issues sit right after that pair's last ACT copy, so
                    # no staging copy is ever delayed).  Pair 2: per class.
                    if i == 2:
                        r0 = 1440 if (ph, pw) == (0, 0) else roff
                        _, nr, ncol = REGION[(ph, pw)]
                        sz = roff + nr * ncol - r0
                        # all pair-2 drains on sync: consecutive DMAs on one
                        # queue still overlap at the engine level, and scalar
                        # then never wedges a drain-issue between pair-2
                        # staging copies (which stalled the stream ~1.7us)
                        nc.sync.dma_start(
                            out=out_ap[2, :, :, r0:r0 + sz],
                            in_=_free_view(stg[0:96], r0, [(1, sz)]))
                    elif (ph, pw) == (0, 1):
                        nc.sync.dma_start(
                            out=out_ap[i, :, :, 0:4560],
                            in_=_free_view(stg[0:96], 0, [(1, 4560)]))
                    elif (ph, pw) == (1, 1):
                        nc.scalar.dma_start(
                            out=out_ap[i, :, :, 4560:9025],
                            in_=_free_view(stg[0:96], 4560, [(1, 4465)]))

    nc.compile()
    return nc


def _get_program():
    if "nc" not in _CACHE:
        _CACHE["nc"] = _build_program()
    return _CACHE["nc"]


def run(x, weight, trace=False):
    from concourse.bass_utils import run_bass_kernel_spmd

    bf16 = _bf16()
    x = np.asarray(x, dtype=np.float32)
    weight = np.asarray(weight, dtype=np.float32)
    in_maps = []
    for j in range(8):
        id0 = min(3 * j, 20)
        xs = np.zeros((3, 4, 8, 4, 49, 49), np.float32)
        for i in range(3):
            for slot in range(4):
                # [c, id, 48, 48]
                xs[i, slot, :, :, :48, :48] = x[0, :, 2 * i + slot, id0:id0 + 4]
        # partition = slot*32 + c*4 + idl  ->  order [i, slot, c, idl, h, w]
        xs = xs.reshape(3, 128, 2401).transpose(1, 0, 2).reshape(128, 7203)
        bands = _build_bands(weight, j).reshape(128, 1152)
        in_maps.append({
            "xs": np.concatenate([bands, xs], axis=1).astype(bf16),
        })
    nc = _get_program()
    res = run_bass_kernel_spmd(nc, in_maps, core_ids=list(range(8)), trace=trace)
    full = np.zeros((1, 8, 6, 47, 95, 95), np.float32)
    for j in range(8):
        nod = min(6, 47 - 6 * j)
        oj = np.asarray(res.results[j]["out"]).astype(np.float32)
        # [pair, fbit, co*6+od, pos] -> [co, frame, od, pos]
        oj = oj.reshape(3, 2, 8, 6, 9025).transpose(2, 0, 1, 3, 4).reshape(
            8, 6, 6, 9025)
        oj = oj[:, :, :nod]  # [8, 6, nod, 9025]
        dst = full[0, :, :, 6 * j:6 * j + nod]
        for (ph, pw), (roff, nr, ncol) in REGION.items():
            dst[..., ph::2, pw::2] = oj[..., roff:roff + nr * ncol].reshape(
                8, 6, nod, nr, ncol)
    return full, res


def kernel(x, weight):
    return run(x, weight)[0]



# revision 44
# speedup vs baseline: 1.0788x; 1.0788x over previous
"""Trainium2 Bass kernel for ConvTranspose4d (T: 3-tap valid conv; D/H/W:
stride-2 k=3 p=1 transposed conv). Self-contained: hardcoded shapes.

x: [1, 8, 8, 24, 48, 48] f32, weight: [8, 8, 3, 3, 3, 3] f32
out: [1, 8, 6, 47, 95, 95] f32

Strategy (8 NeuronCores, data-parallel over D):
  - Core j computes output od = 6j..6j+5 (core 7 drops od 47); needs input
    slices id0..id0+3 (id0 = min(3j, 20)).
  - Temporal 3-tap conv and D-axis stride-2 transposed conv fold into the
    matmul stationary operand as a banded weight matrix:
      lhsT[K=128=(slot4, cin8, id4), M=(fbit2 x 48=(cout8*6+od))]
    where slot = kt + fbit holds temporal plane 2i+slot of frame-pair i.
  - All I/O is bf16 (host converts): one flat input DRAM tensor
    [128, bands(1152) | 3 x plane(2401)], loaded in range-split pieces
    strictly serialized on the sync HWDGE queue (concurrent queues contend
    per-engine at packet granularity).  Bands live in their own SBUF tile
    (lhsT+rhs from one tensor costs ~40ns/MM in port conflicts).
  - N=512 warm-up matmuls open the HAM clock gate (K=8/8) before the real
    stream, which then runs gap-free at ~206ns/MM (N=480, warm floor).
  - H/W parities are 4 output classes (ph, pw); each accumulates 1/2/2/4
    shifted-view taps in PSUM (kh = ph - 2*dh + 1).
  - Staging is PARITY-PLANAR: each class region is contiguous per
    partition (DVE for ph=0, ACT for ph=1; final pair alternates).  Output
    streams out DURING compute: pairs 0/1 drain per ph-half as soon as the
    half is staged, pair 2 per class with the last class pre-drained at
    chunk 2, so the post-stream tail is only ~170 KB + completion.
"""
import numpy as np

COMPUTE = "bfloat16"

TAPS = {
    (0, 0): [(0, 0)],
    (0, 1): [(0, 0), (0, 1)],
    (1, 0): [(0, 0), (1, 0)],
    (1, 1): [(0, 0), (0, 1), (1, 0), (1, 1)],
}
TAP_LIST = [(ph, pw, dh, dw) for (ph, pw), tl in TAPS.items() for (dh, dw) in tl]
CHUNK_START = [0, 10, 20, 30, 40]
CHUNK_N = [10, 10, 10, 10, 8]
PAIRS = [(0, 1), (2, 3), (4,)]
# class -> (region offset in stg, rows, cols)
REGION = {
    (0, 0): (0, 48, 48),
    (0, 1): (2304, 48, 47),
    (1, 0): (4560, 47, 48),
    (1, 1): (6816, 47, 47),
}

_CACHE = {}


def _bf16():
    import ml_dtypes
    return ml_dtypes.bfloat16


def _build_bands(W, j):
    """W: [cin8, cout8, kt3, kd3, kh3, kw3] -> [128, 9, 128] f32.
    K row = slot*32 + cin*4 + id (slot = kt + fbit);
    M col = fbit*48 + cout*6 + od (od 0..5; cols 96..127 zero)."""
    id0 = min(3 * j, 20)
    B = np.zeros((128, 9, 128), np.float32)
    ci = np.arange(8)
    co = np.arange(8)
    for t, (ph, pw, dh, dw) in enumerate(TAP_LIST):
        kh = ph - 2 * dh + 1
        kw = pw - 2 * dw + 1
        for fbit in range(2):
            for kt in range(3):
                slot = kt + fbit
                for idl in range(4):
                    for od in range(6):
                        od_g = 6 * j + od
                        if od_g > 46:
                            continue
                        kd = od_g - 2 * (id0 + idl) + 1
                        if not (0 <= kd <= 2):
                            continue
                        krow = slot * 32 + ci * 4 + idl
                        mcol = fbit * 48 + co * 6 + od
                        B[krow[:, None], t, mcol[None, :]] = W[:, :, kt, kd, kh, kw]
    return B


def _free_view(base, off, dims):
    """Hand-built AP: keep base's partition dim, replace free dims with
    [(step, count), ...] (element units) at extra offset `off`."""
    a = base.copy()
    v = a.ap
    part = v.to_list()[0]
    v.clear()
    v.append(part)
    for sc in dims:
        v.append(list(sc))
    a.ap = v
    a.offset = a.offset + off
    return a


def _build_program():
    import concourse.bacc as bacc
    import concourse.tile as tile
    from concourse import mybir

    f32 = mybir.dt.float32
    bf16 = mybir.dt.bfloat16

    nc = bacc.Bacc("TRN2", target_bir_lowering=False, debug=False)
    # flat per-partition layout [bands(1152) | p0(2401) | p1(2401) | p2(2401)]
    # -> one SBUF tile, big DMA descriptors, range-split loads so the data
    # matmul #0 needs (bands + plane-0 top rows) lands first.
    xs_ap = nc.dram_tensor("xs", [128, 8355], bf16, kind="ExternalInput").ap()
    # [pair, fbit, co*6+od, pos] -- matches stg partition order so each pair
    # drains as ONE big DMA (96 partitions x 18 KB).
    out_ap = nc.dram_tensor("out", [3, 2, 48, 9025], bf16, kind="ExternalOutput").ap()

    with tile.TileContext(nc, trace_sim=False) as tc:
        with (
            tc.tile_pool(name="bp", bufs=1) as bp,
            tc.tile_pool(name="sp", bufs=3) as sp,
            tc.tile_pool(name="ps", bufs=8, space="PSUM") as ps,
        ):
            # PE warm-up: N=512 dummy matmuls (~430ns each, ~100% PE-busy) so
            # the HAM clock-gate reaches K=8/8 before the real stream starts.
            # Small-N warmups measurably never warm it (too many dispatch
            # bubbles inside the 4096-cycle activity window).
            dz = bp.tile([128, 512], bf16)
            nc.gpsimd.memset(dz[:], 0.0)
            wps = ps.tile([128, 512], f32, name="warm", tag="ps")
            for _ in range(10):
                nc.tensor.matmul(wps[:, 0:512], dz[:, 0:128], dz[:],
                                 start=True, stop=True)

            # bands in their OWN SBUF tensor: lhsT and rhs from the same
            # tensor costs ~40ns/MM in SBUF port conflicts (measured).
            bt = bp.tile([128, 1152], bf16)
            xa = bp.tile([128, 7203], bf16)
            # Strictly serialize input on the sync HWDGE queue -- concurrent
            # queues contend per-engine at packet granularity (measured:
            # 301KB crawling at 50GB/s behind a parallel queue).  Order:
            # bands + plane-0 top rows (feeds MM0/chunks 0-1), rest of
            # plane 0, planes 1-2 (needed ~10us later).
            nc.sync.dma_start(out=bt[:], in_=xs_ap[:, 0:1152])
            nc.sync.dma_start(out=xa[:, 0:1225], in_=xs_ap[:, 1152:2377])
            nc.sync.dma_start(out=xa[:, 1225:2401], in_=xs_ap[:, 2377:3553])
            # 1-element WAW overlap with the previous piece: forces B to
            # start only after plane 0 has fully landed (otherwise their
            # descriptors interleave per-engine and plane 0 crawls).
            nc.sync.dma_start(out=xa[:, 2400:7203], in_=xs_ap[:, 3552:8355])

            for i in range(3):
                poff = i * 2401
                stg = sp.tile([128, 9025], bf16, name=f"stg{i}", tag="stg")
                corder = list(TAPS) if i < 2 else list(TAPS)[::-1]
                if i == 0:
                    # chunks 0-1 of all classes first: they only touch
                    # plane-0 rows 0-21 (the first input piece), giving the
                    # second piece ~3.7us of slack to land -- robust against
                    # slow-DMA runs that otherwise stall the stream AND
                    # re-throttle the HAM clock gate (measured +4us outlier).
                    # Then class-major so the ph-half drains still fire early.
                    worklist = [(cls, c) for c in (0, 1) for cls in corder]
                    worklist += [(cls, c) for cls in corder for c in (2, 3, 4)]
                else:
                    worklist = [(cls, c) for cls in corder for c in range(5)]
                for (ph, pw), c in worklist:
                    taps = TAPS[(ph, pw)]
                    roff, _, nmw = REGION[(ph, pw)]
                    if True:
                        pt = ps.tile([128, 512], f32, name="ps", tag="ps")
                        mh0, nmh = CHUNK_START[c], CHUNK_N[c]
                        for ti, (dh, dw) in enumerate(taps):
                            t_idx = TAP_LIST.index((ph, pw, dh, dw))
                            lhsT = bt[:, t_idx * 128:(t_idx + 1) * 128]
                            rhs = _free_view(
                                xa[:], poff + (mh0 + dh) * 49 + dw,
                                [(49, nmh), (1, 48)])
                            nc.tensor.matmul(
                                pt[:, 0:nmh * 48], lhsT, rhs,
                                start=(ti == 0), stop=(ti == len(taps) - 1),
                            )
                        # contiguous copy PSUM -> class-planar staging; the
                        # final pair alternates DVE/ACT per chunk so copies
                        # chase the matmuls
                        nmh_c = nmh if c < 4 else CHUNK_N[4] - ph
                        src = _free_view(pt[0:96], 0, [(48, nmh_c), (1, nmw)])
                        doff = roff + CHUNK_START[c] * nmw
                        dst = _free_view(stg[0:96], doff,
                                         [(nmw, nmh_c), (1, nmw)])
                        use_dve = (c % 2 == 0) if i == 2 else (ph == 0)
                        if use_dve:
                            nc.vector.tensor_copy(dst, src)
                        else:
                            nc.scalar.copy(dst, src)
                        if i == 2 and (ph, pw) == (0, 0) and c == 2:
                            # pre-drain rows 0-29 of the very last class so
                            # the post-stream tail is only ~170 KB
                            nc.sync.dma_start(
                                out=out_ap[2, :, :, 0:1440],
                                in_=_free_view(stg[0:96], 0, [(1, 1440)]))
                    # Drain as soon as a region is staged (c==4 completes a
                    # class), with the DMA issue placed before later copies
                    # in its engine's FIFO queue.  Pairs 0/1: per ph-half
                    # (the only two scalar issues sit right after that
                    # pair's last ACT copy, so no staging copy is ever
                    # delayed).  Pair 2: per class, all on sync (consecutive
                    # DMAs on one queue still overlap at the engine level,
                    # and scalar never wedges a drain-issue between pair-2
                    # staging copies).
                    if c < 4:
                        pass
                    elif i == 2:
                        r0 = 1440 if (ph, pw) == (0, 0) else roff
                        _, nr, ncol = REGION[(ph, pw)]
                        sz = roff + nr * ncol - r0
                        nc.sync.dma_start(
                            out=out_ap[2, :, :, r0:r0 + sz],
                            in_=_free_view(stg[0:96], r0, [(1, sz)]))
                    elif (ph, pw) == (0, 1):
                        nc.sync.dma_start(
                            out=out_ap[i, :, :, 0:4560],
                            in_=_free_view(stg[0:96], 0, [(1, 4560)]))
                    elif (ph, pw) == (1, 1):
                        nc.scalar.dma_start(
                            out=out_ap[i, :, :, 4560:9025],
                            in_=_free_view(stg[0:96], 4560, [(1, 4465)]))

    nc.compile()
    return nc


def _get_program():
    if "nc" not in _CACHE:
        _CACHE["nc"] = _build_program()
    return _CACHE["nc"]


def run(x, weight, trace=False):
    from concourse.bass_utils import run_bass_kernel_spmd

    bf16 = _bf16()
    x = np.asarray(x, dtype=np.float32)
    weight = np.asarray(weight, dtype=np.float32)
    in_maps = []
    for j in range(8):
        id0 = min(3 * j, 20)
        xs = np.zeros((3, 4, 8, 4, 49, 49), np.float32)
        for i in range(3):
            for slot in range(4):
                # [c, id, 48, 48]
                xs[i, slot, :, :, :48, :48] = x[0, :, 2 * i + slot, id0:id0 + 4]
        # partition = slot*32 + c*4 + idl  ->  order [i, slot, c, idl, h, w]
        xs = xs.reshape(3, 128, 2401).transpose(1, 0, 2).reshape(128, 7203)
        bands = _build_bands(weight, j).reshape(128, 1152)
        in_maps.append({
            "xs": np.concatenate([bands, xs], axis=1).astype(bf16),
        })
    nc = _get_program()
    res = run_bass_kernel_spmd(nc, in_maps, core_ids=list(range(8)), trace=trace)
    full = np.zeros((1, 8, 6, 47, 95, 95), np.float32)
    for j in range(8):
        nod = min(6, 47 - 6 * j)
        oj = np.asarray(res.results[j]["out"]).astype(np.float32)
        # [pair, fbit, co*6+od, pos] -> [co, frame, od, pos]
        oj = oj.reshape(3, 2, 8, 6, 9025).transpose(2, 0, 1, 3, 4).reshape(
            8, 6, 6, 9025)
        oj = oj[:, :, :nod]  # [8, 6, nod, 9025]
        dst = full[0, :, :, 6 * j:6 * j + nod]
        for (ph, pw), (roff, nr, ncol) in REGION.items():
            dst[..., ph::2, pw::2] = oj[..., roff:roff + nr * ncol].reshape(
                8, 6, nod, nr, ncol)
    return full, res


def kernel(x, weight):
    return run(x, weight)[0]



# revision 57
# speedup vs baseline: 1.1411x; 1.0577x over previous
"""Trainium2 Bass kernel for ConvTranspose4d (T: 3-tap valid conv; D/H/W:
stride-2 k=3 p=1 transposed conv). Self-contained: hardcoded shapes.

x: [1, 8, 8, 24, 48, 48] f32, weight: [8, 8, 3, 3, 3, 3] f32
out: [1, 8, 6, 47, 95, 95] f32

Strategy (8 NeuronCores, data-parallel over D):
  - Core j computes output od = 6j..6j+5 (core 7 drops od 47); needs input
    slices id0..id0+3 (id0 = min(3j, 20)).
  - Temporal 3-tap conv and D-axis stride-2 transposed conv fold into the
    matmul stationary operand as a banded weight matrix:
      lhsT[K=128=(slot4, cin8, id4), M=(fbit2 x 48=(cout8*6+od))]
    where slot = kt + fbit holds temporal plane 2i+slot of frame-pair i.
  - All I/O is bf16 (host converts): one flat input DRAM tensor
    [128, bands(1152) | 3 x plane(2401)], loaded in range-split pieces
    strictly serialized on the sync HWDGE queue (concurrent queues contend
    per-engine at packet granularity).  Bands live in their own SBUF tile
    (lhsT+rhs from one tensor costs ~40ns/MM in port conflicts).
  - N=512 warm-up matmuls open the HAM clock gate (K=8/8) before the real
    stream, which then runs gap-free at ~206ns/MM (N=480, warm floor).
  - H/W parities are 4 output classes (ph, pw); each accumulates 1/2/2/4
    shifted-view taps in PSUM (kh = ph - 2*dh + 1).
  - Staging is PARITY-PLANAR: each class region is contiguous per
    partition (DVE for ph=0, ACT for ph=1; final pair alternates).  Output
    streams out DURING compute: pairs 0/1 drain per ph-half as soon as the
    half is staged, pair 2 per class with the last class pre-drained at
    chunk 2, so the post-stream tail is only ~150 KB + completion.
"""
import numpy as np

COMPUTE = "bfloat16"

TAPS = {
    (0, 0): [(0, 0)],
    (0, 1): [(0, 0), (0, 1)],
    (1, 0): [(0, 0), (1, 0)],
    (1, 1): [(0, 0), (0, 1), (1, 0), (1, 1)],
}
TAP_LIST = [(ph, pw, dh, dw) for (ph, pw), tl in TAPS.items() for (dh, dw) in tl]
CHUNK_START = [0, 10, 20, 30, 40]
CHUNK_N = [10, 10, 10, 10, 8]
PAIRS = [(0, 1), (2, 3), (4,)]
# class -> (region offset in stg, rows, cols)
REGION = {
    (0, 0): (0, 48, 48),
    (0, 1): (2304, 48, 47),
    (1, 0): (4560, 47, 48),
    (1, 1): (6816, 47, 47),
}

_CACHE = {}


def _bf16():
    import ml_dtypes
    return ml_dtypes.bfloat16


def _build_bands(W, j):
    """W: [cin8, cout8, kt3, kd3, kh3, kw3] -> [128, 9, 128] f32.
    K row = slot*32 + cin*4 + id (slot = kt + fbit);
    M col = fbit*48 + cout*6 + od (od 0..5; cols 96..127 zero)."""
    id0 = min(3 * j, 20)
    B = np.zeros((128, 9, 128), np.float32)
    ci = np.arange(8)
    co = np.arange(8)
    for t, (ph, pw, dh, dw) in enumerate(TAP_LIST):
        kh = ph - 2 * dh + 1
        kw = pw - 2 * dw + 1
        for fbit in range(2):
            for kt in range(3):
                slot = kt + fbit
                for idl in range(4):
                    for od in range(6):
                        od_g = 6 * j + od
                        if od_g > 46:
                            continue
                        kd = od_g - 2 * (id0 + idl) + 1
                        if not (0 <= kd <= 2):
                            continue
                        krow = slot * 32 + ci * 4 + idl
                        mcol = fbit * 48 + co * 6 + od
                        B[krow[:, None], t, mcol[None, :]] = W[:, :, kt, kd, kh, kw]
    return B


def _free_view(base, off, dims):
    """Hand-built AP: keep base's partition dim, replace free dims with
    [(step, count), ...] (element units) at extra offset `off`."""
    a = base.copy()
    v = a.ap
    part = v.to_list()[0]
    v.clear()
    v.append(part)
    for sc in dims:
        v.append(list(sc))
    a.ap = v
    a.offset = a.offset + off
    return a


def _build_program():
    import concourse.bacc as bacc
    import concourse.tile as tile
    from concourse import mybir

    f32 = mybir.dt.float32
    bf16 = mybir.dt.bfloat16

    nc = bacc.Bacc("TRN2", target_bir_lowering=False, debug=False)
    # flat per-partition layout [bands(1152) | p0(2401) | p1(2401) | p2(2401)]
    # -> one SBUF tile, big DMA descriptors, range-split loads so the data
    # matmul #0 needs (bands + plane-0 top rows) lands first.
    xs_ap = nc.dram_tensor("xs", [128, 8355], bf16, kind="ExternalInput").ap()
    # [pair, fbit, co*6+od, pos] -- matches stg partition order so each pair
    # drains as ONE big DMA (96 partitions x 18 KB).
    out_ap = nc.dram_tensor("out", [3, 2, 48, 9025], bf16, kind="ExternalOutput").ap()

    with tile.TileContext(nc, trace_sim=False) as tc:
        with (
            tc.tile_pool(name="bp", bufs=1) as bp,
            tc.tile_pool(name="sp", bufs=3) as sp,
            tc.tile_pool(name="ps", bufs=8, space="PSUM") as ps,
        ):
            # PE warm-up: N=512 dummy matmuls (~430ns each, ~100% PE-busy) so
            # the HAM clock-gate reaches K=8/8 before the real stream starts.
            # Small-N warmups measurably never warm it (too many dispatch
            # bubbles inside the 4096-cycle activity window).
            dz = bp.tile([128, 512], bf16)
            nc.gpsimd.memset(dz[:], 0.0)
            wps = ps.tile([128, 512], f32, name="warm", tag="ps")
            for _ in range(9):
                nc.tensor.matmul(wps[:, 0:512], dz[:, 0:128], dz[:],
                                 start=True, stop=True)

            # bands in their OWN SBUF tensor: lhsT and rhs from the same
            # tensor costs ~40ns/MM in SBUF port conflicts (measured).
            bt = bp.tile([128, 1152], bf16)
            xa = bp.tile([128, 7203], bf16)
            # Strictly serialize input on the sync HWDGE queue -- concurrent
            # queues contend per-engine at packet granularity (measured:
            # 301KB crawling at 50GB/s behind a parallel queue).  Order:
            # bands + plane-0 top rows (feeds MM0/chunks 0-1), rest of
            # plane 0, planes 1-2 (needed ~10us later).
            # bands split: taps 0-4 feed classes (0,0),(0,1),(1,0) whose
            # chunk-0/1 matmuls fill the first ~2us of the stream; the
            # (1,1) taps arrive while those run.  This pulls plane-0's
            # first piece (and MM0) ~1us earlier.
            nc.sync.dma_start(out=bt[:, 0:640], in_=xs_ap[:, 0:640])
            nc.sync.dma_start(out=xa[:, 0:637], in_=xs_ap[:, 1152:1789])
            nc.sync.dma_start(out=bt[:, 640:1152], in_=xs_ap[:, 640:1152])
            nc.sync.dma_start(out=xa[:, 637:1225], in_=xs_ap[:, 1789:2377])
            nc.sync.dma_start(out=xa[:, 1225:2401], in_=xs_ap[:, 2377:3553])
            # 1-element WAW overlap with the previous piece: forces B to
            # start only after plane 0 has fully landed (otherwise their
            # descriptors interleave per-engine and plane 0 crawls).
            nc.sync.dma_start(out=xa[:, 2400:7203], in_=xs_ap[:, 3552:8355])

            for i in range(3):
                poff = i * 2401
                stg = sp.tile([128, 9025], bf16, name=f"stg{i}", tag="stg")
                # normal class order for ALL pairs: ending a pair with the
                # 1-tap class stalls the stream (206ns/chunk matmuls outrun
                # the ~620ns staging copies -> PSUM-bank starvation, measured
                # 3x565ns gaps); the 4-tap class last (824ns/chunk) lets the
                # copies keep pace.
                corder = list(TAPS)
                if i == 0:
                    # chunks 0-1 of all classes first: they only touch
                    # plane-0 rows 0-21 (the first input piece), giving the
                    # second piece ~3.7us of slack to land -- robust against
                    # slow-DMA runs that otherwise stall the stream AND
                    # re-throttle the HAM clock gate (measured +4us outlier).
                    # Then class-major so the ph-half drains still fire early.
                    worklist = [(cls, c) for c in (0, 1) for cls in corder]
                    worklist += [(cls, c) for cls in corder for c in (2, 3, 4)]
                else:
                    worklist = [(cls, c) for cls in corder for c in range(5)]
                for (ph, pw), c in worklist:
                    taps = TAPS[(ph, pw)]
                    roff, _, nmw = REGION[(ph, pw)]
                    if True:
                        pt = ps.tile([128, 512], f32, name="ps", tag="ps")
                        mh0, nmh = CHUNK_START[c], CHUNK_N[c]
                        # stream only VALID output columns: ph=1 classes have
                        # 47 rows (trim chunk 4 to 7) and pw=1 classes 47
                        # cols (trim the w-window) -- saves ~1800 columns
                        nmh_m = nmh - (1 if (c == 4 and ph == 1) else 0)
                        for ti, (dh, dw) in enumerate(taps):
                            t_idx = TAP_LIST.index((ph, pw, dh, dw))
                            lhsT = bt[:, t_idx * 128:(t_idx + 1) * 128]
                            rhs = _free_view(
                                xa[:], poff + (mh0 + dh) * 49 + dw,
                                [(49, nmh_m), (1, nmw)])
                            nc.tensor.matmul(
                                pt[:, 0:nmh_m * nmw], lhsT, rhs,
                                start=(ti == 0), stop=(ti == len(taps) - 1),
                            )
                        # contiguous copy PSUM -> class-planar staging; the
                        # final pair alternates DVE/ACT per chunk so copies
                        # chase the matmuls
                        nmh_c = nmh if c < 4 else CHUNK_N[4] - ph
                        src = _free_view(pt[0:96], 0, [(nmw, nmh_c), (1, nmw)])
                        doff = roff + CHUNK_START[c] * nmw
                        dst = _free_view(stg[0:96], doff,
                                         [(nmw, nmh_c), (1, nmw)])
                        use_dve = (c % 2 == 0) if i == 2 else (ph == 0)
                        if use_dve:
                            nc.vector.tensor_copy(dst, src)
                        else:
                            nc.scalar.copy(dst, src)
                        if i == 2 and (ph, pw) == (1, 1) and c == 2:
                            # pre-drain rows 0-29 of the very last class
                            nc.sync.dma_start(
                                out=out_ap[2, :, :, 6816:8226],
                                in_=_free_view(stg[0:96], 6816, [(1, 1410)]))
                        if i == 2 and (ph, pw) == (1, 1) and c == 3:
                            # ...and rows 30-39, so the post-stream tail is
                            # only the 63 KB of rows 40-46
                            nc.sync.dma_start(
                                out=out_ap[2, :, :, 8226:8696],
                                in_=_free_view(stg[0:96], 8226, [(1, 470)]))
                    # Drain as soon as a region is staged (c==4 completes a
                    # class), with the DMA issue placed before later copies
                    # in its engine's FIFO queue.  Pairs 0/1: per ph-half
                    # (the only two scalar issues sit right after that
                    # pair's last ACT copy, so no staging copy is ever
                    # delayed).  Pair 2: per class, all on sync (consecutive
                    # DMAs on one queue still overlap at the engine level,
                    # and scalar never wedges a drain-issue between pair-2
                    # staging copies).
                    if c < 4:
                        pass
                    elif i == 2:
                        r0 = 8696 if (ph, pw) == (1, 1) else roff
                        _, nr, ncol = REGION[(ph, pw)]
                        sz = roff + nr * ncol - r0
                        # the final ~150KB piece rides scalar (idle by then)
                        # so it overlaps the rows-0-29 piece on sync instead
                        # of queueing behind it
                        eng = nc.scalar if (ph, pw) == (1, 1) else nc.sync
                        eng.dma_start(
                            out=out_ap[2, :, :, r0:r0 + sz],
                            in_=_free_view(stg[0:96], r0, [(1, sz)]))
                    elif (ph, pw) == (0, 1):
                        nc.sync.dma_start(
                            out=out_ap[i, :, :, 0:4560],
                            in_=_free_view(stg[0:96], 0, [(1, 4560)]))
                    elif (ph, pw) == (1, 1):
                        nc.scalar.dma_start(
                            out=out_ap[i, :, :, 4560:9025],
                            in_=_free_view(stg[0:96], 4560, [(1, 4465)]))

    nc.compile()
    return nc


def _get_program():
    if "nc" not in _CACHE:
        _CACHE["nc"] = _build_program()
    return _CACHE["nc"]


def run(x, weight, trace=False):
    from concourse.bass_utils import run_bass_kernel_spmd

    bf16 = _bf16()
    x = np.asarray(x, dtype=np.float32)
    weight = np.asarray(weight, dtype=np.float32)
    in_maps = []
    for j in range(8):
        id0 = min(3 * j, 20)
        xs = np.zeros((3, 4, 8, 4, 49, 49), np.float32)
        for i in range(3):
            for slot in range(4):
                # [c, id, 48, 48]
                xs[i, slot, :, :, :48, :48] = x[0, :, 2 * i + slot, id0:id0 + 4]
        # partition = slot*32 + c*4 + idl  ->  order [i, slot, c, idl, h, w]
        xs = xs.reshape(3, 128, 2401).transpose(1, 0, 2).reshape(128, 7203)
        bands = _build_bands(weight, j).reshape(128, 1152)
        in_maps.append({
            "xs": np.concatenate([bands, xs], axis=1).astype(bf16),
        })
    nc = _get_program()
    res = run_bass_kernel_spmd(nc, in_maps, core_ids=list(range(8)), trace=trace)
    full = np.zeros((1, 8, 6, 47, 95, 95), np.float32)
    for j in range(8):
        nod = min(6, 47 - 6 * j)
        oj = np.asarray(res.results[j]["out"]).astype(np.float32)
        # [pair, fbit, co*6+od, pos] -> [co, frame, od, pos]
        oj = oj.reshape(3, 2, 8, 6, 9025).transpose(2, 0, 1, 3, 4).reshape(
            8, 6, 6, 9025)
        oj = oj[:, :, :nod]  # [8, 6, nod, 9025]
        dst = full[0, :, :, 6 * j:6 * j + nod]
        for (ph, pw), (roff, nr, ncol) in REGION.items():
            dst[..., ph::2, pw::2] = oj[..., roff:roff + nr * ncol].reshape(
                8, 6, nod, nr, ncol)
    return full, res


def kernel(x, weight):
    return run(x, weight)[0]

